# revision 19
# baseline (speedup 1.0000x reference)
"""Trainium2 Bass kernel for nn_Interpolator (quadratic-form kernel interpolation).

Math (T=8192 targets, C=8192 contexts, D=64, DY=32):
    S = W + W^T
    scores[t,c] = (z_t - z_c)^T W (z_t - z_c)
                = q_tt[t] + q_cc[c] - z_t^T S z_c
    theta = exp(-scores);  out = (theta @ y_context) / theta.sum(-1, keepdim)

q_tt[t] scales whole theta rows and cancels in the normalization -> dropped.
q_cc[c] = 0.5 * z_c^T S z_c is folded into the main matmul contraction:
the stationary operand LC has 128 rows: rows 0..63 = zc^T, rows 64..127 =
0.5*(zc .* (S zc)); the moving operand RT has rows 0..63 = S^T z_t and
rows 64..127 = -1. A single K=128 fp16 matmul then yields cross - q_cc
directly (matmul cost depends only on moving columns, so the fold is free).

Sharding: data-parallel over targets; each of the 8 cores takes T/8 = 1024
targets and the full context set.

Per-core device program, v5. The ACT exp stream is the roofline: measured,
back-to-back 1024-wide ACTIVATEs run at ~996 ns each (pipe-fill overlaps),
beating any wider-instruction scheme the PSUM bank budget allows. The loop
is a 3-tile rotation of [128,1024] score tiles (6 banks) whose exps run
back-to-back; NOTHING else touches those tiles mid-loop:
  - ALL 16 zs pieces (LC rows 64..127) are produced before mm2 needs the
    o2 banks: pieces 0-1 staged in the rotation tiles pre-loop, pieces
    2-15 in seven rounds staged in the o2 banks' rows 0:64 (DVE writes
    LC directly; consumers are 10+ windows away, so no SBUF-DMA bounce
    and zero coupling into the exp stream).
  - mm2 therefore starts late: theta tiles live in a 10-deep ring, and
    column-tiled PAIRS (chunk c -> band 0 = partitions 0:33, chunk c+1 ->
    band 64:97, concurrent in PE column-groups) are emitted from chunk 8
    on, catching up at 3/2/1 pairs per even chunk. The bands are summed
    on the host (no cross-partition add on device).
  - input DMAs ride the two hardware DGE rings in first-use order:
    scalar ring W -> y h0; sync ring zt -> zc[0:512] -> zc[512:1024] ->
    zc bulk -> y h1. Dep-free filler matmuls into o2 keep PE busy through
    the DMA phase (HAM 8/8 before chunk 0).
  - RT = S^T zt is staged in PS2 and cast by ONE ACT copy (ACT is idle
    pre-loop; engine writes to one tile serialize anyway).
  - output: one [33, TL] DMA per band per ring (33 rows fan across 11
    DMA engines; a 97-row transfer lands on ONE engine - measured);
    chunk 63's exp is split 2x512 so the final pair starts early.
Host: shard/transpose/cast inputs (layout only); gather, sum the two
bands, divide numerator rows by the denominator row.
"""

import ml_dtypes
import numpy as np

import concourse.bacc as bacc
import concourse.bass as bass
import concourse.mybir as mybir
import concourse.tile as tile
from concourse.bass_utils import run_bass_kernel_spmd

F32 = mybir.dt.float32
F16 = mybir.dt.float16
BF16 = mybir.dt.bfloat16

T, C, D, DY = 8192, 8192, 64, 32
NCORES = 8
TL = T // NCORES          # 1024 targets per core
NCHUNK = C // 128         # 64 context chunks of 128
NPIECE = C // 512         # 16 zs pieces of 512 contexts
NWARM = 4
NFILL = 6
NTH = 20                  # theta ring depth (max mm2 lag 16 + safety)
OB = 64                   # second mm2 band base partition (column-group 2)
# consecutive offload pairs: one conservative-WAR gap per PAIR of skipped
# ACT exps (a lone skip pays the same ~2.2us chain for half the saving)
DVE_EXP = {20, 21, 27, 28, 34, 35, 41, 42, 48, 49, 55, 56}
SCH_A = float(2 ** 23 / np.log(2))          # Schraudolph scale
SCH_B = float(127 * 2 ** 23 - 486411)       # Schraudolph bias (minimax C)


def _build_kernel_body(tc: tile.TileContext):
    nc = tc.nc
    Exp = mybir.ActivationFunctionType.Exp

    wwt_d = nc.dram_tensor("wwt", [D, 2 * D], F32, kind="ExternalInput")
    zt_d = nc.dram_tensor("ztt", [D, TL], F16, kind="ExternalInput")
    zca_d = nc.dram_tensor("zca", [D, 512], F16, kind="ExternalInput")
    zcb_d = nc.dram_tensor("zcb", [D, 512], F16, kind="ExternalInput")
    zcv_d = nc.dram_tensor("zcv", [D, 3584], F16, kind="ExternalInput")
    zct_d = nc.dram_tensor("zct", [D, 3584], F16, kind="ExternalInput")
    y_d = nc.dram_tensor("yck", [128, NCHUNK * DY], BF16, kind="ExternalInput")
    out0_d = nc.dram_tensor("out0", [DY + 1, TL], F32, kind="ExternalOutput")
    out1_d = nc.dram_tensor("out1", [DY + 1, TL], F32, kind="ExternalOutput")

    with (
        tc.tile_pool(name="sb", bufs=1) as sb,
        tc.tile_pool(name="pp", bufs=1, space="PSUM") as pp,
    ):
        # ---- resident SBUF slabs ----
        LC = sb.tile([128, C], F16, name="lc")
        RT = sb.tile([128, TL], F16, name="rt")
        ZT = sb.tile([D, TL], F16, name="zt")
        YT = sb.tile([128, NCHUNK * DY], BF16, name="yt")
        YA = sb.tile([128, NCHUNK, DY + 1], BF16, name="ya")
        WW = sb.tile([D, 2 * D], F32, name="ww")
        SS = sb.tile([D, D], F16, name="ss")
        SSH = sb.tile([D, D], F16, name="ssh")
        THS = [sb.tile([128, TL], BF16, name=f"th{i}") for i in range(NTH)]
        OSB = sb.tile([OB + DY + 1, TL], F32, name="osb")
        WRM = sb.tile([128, 512], BF16, name="wrm")
        TMPA = [sb.tile([128, TL], F32, name=f"tmpa{i}") for i in range(2)]
        TMPB = sb.tile([128, TL], mybir.dt.uint32, name="tmpb")
        EXD = sb.tile([D, 1], F32, name="exd")

        # ---- PSUM: 3 rotating score tiles (6 banks) + o2 (2 banks) ----
        PS0 = pp.tile([128, TL], F32, tag="ring0", name="ps0")
        PS1 = pp.tile([128, TL], F32, tag="ring1", name="ps1")
        PS2 = pp.tile([128, TL], F32, tag="ring2", name="ps2")
        o2 = pp.tile([128, TL], F32, tag="o2", name="o2")
        PSC = [PS0, PS1, PS2]

        # ---- input DMAs on the two HWDGE rings, first-use order ----
        half_y = NCHUNK * DY // 2
        nc.scalar.dma_start(out=WW, in_=wwt_d.ap())
        nc.scalar.dma_start(out=LC[:D, 0:512], in_=zca_d.ap())
        nc.scalar.dma_start(out=YT[:, :half_y], in_=y_d.ap()[:, :half_y])
        nc.sync.dma_start(out=ZT, in_=zt_d.ap())
        nc.sync.dma_start(out=LC[:D, 512:1024], in_=zcb_d.ap())
        nc.sync.dma_start(out=LC[:D, 1024:4608], in_=zcv_d.ap())
        nc.sync.dma_start(out=LC[:D, 4608:8192], in_=zct_d.ap())
        nc.sync.dma_start(out=YT[:, half_y:], in_=y_d.ap()[:, half_y:])

        # exp-table preload (so the first chunk ACTIVATE is cheap)
        nc.vector.memset(EXD, 0.0)
        nc.scalar.activation(EXD, EXD, Exp)

        # ---- PE warm-up + fillers: keep PE busy through the DMA phase
        # (HAM -> 8/8); the scheduler slots real matmuls in as ready ----
        nc.vector.memset(WRM, 0.5)
        for i in range(NWARM):
            nc.tensor.matmul(
                PSC[i % 2][:, (i // 2) * 512 : (i // 2 + 1) * 512],
                WRM[:, 0:128], WRM, start=True, stop=True,
            )
        for i in range(NFILL):
            nc.tensor.matmul(
                o2[:, (i % 2) * 512 : (i % 2 + 1) * 512], WRM[:, 0:128], WRM,
                start=True, stop=True,
            )

        # ---- DVE prelude chain ----
        nc.vector.memset(RT[D:128, :], -1.0)
        nc.vector.tensor_add(SS, WW[:, 0:D], WW[:, D : 2 * D])   # fp16 S
        nc.vector.tensor_scalar_mul(SSH, SS, 0.5)                # fp16 S/2

        # ---- RT rows 0..63 = S^T zt staged in PS2, one ACT cast (ACT is
        # idle pre-loop); chunks 0/1 never wait on PS2 ----
        for h in range(2):
            sl = slice(h * 512, (h + 1) * 512)
            nc.tensor.matmul(PS2[:D, sl], SS, ZT[:, sl], start=True, stop=True)
        nc.scalar.copy(RT[:D, 0:512], PS2[:D, 0:512])
        nc.scalar.copy(RT[:D, 512:1024], PS2[:D, 512:1024])

        # ---- zs pieces 0,1 staged in PS0/PS1 region [512:1024]; DVE
        # writes LC rows 64:128 directly ----
        for k in range(2):
            sl = slice(512 * k, 512 * (k + 1))
            nc.tensor.matmul(PSC[k][:D, 512:1024], SSH, LC[:D, sl],
                             start=True, stop=True)
            nc.vector.tensor_mul(LC[D:128, sl], PSC[k][:D, 512:1024],
                                 LC[:D, sl])

        # zs piece k (2..15) staged in o2 rows 0:64, emitted at chunk
        # k-2 so the alternating-region WAR chain always has two windows
        # of slack and never head-of-line-blocks the PE queue; DVE
        # writes LC rows 64:128 directly (consumers are 8+ windows out)
        def zs_piece(k):
            sl = slice(512 * k, 512 * (k + 1))
            st = slice((k % 2) * 512, (k % 2 + 1) * 512)
            nc.tensor.matmul(o2[:D, st], SSH, LC[:D, sl], start=True,
                             stop=True)
            nc.vector.tensor_mul(LC[D:128, sl], o2[:D, st], LC[:D, sl])

        # y_aug piece q (16 chunks): [128, 16, 33]; col 32 = 1.0
        nc.vector.memset(YA[:, :, DY : DY + 1], 1.0)
        qy = NCHUNK // 4 * DY

        def ya_piece(q, eng):
            eng(
                YA[:, q * 16 : (q + 1) * 16, 0:DY],
                YT[:, q * qy : (q + 1) * qy].rearrange("p (j d) -> p j d", d=DY),
            )

        ya_piece(0, nc.vector.tensor_copy)   # DVE; ACT stays on the cast chain

        def score_mms(P, c):
            lhsT = LC[:, c * 128 : (c + 1) * 128]
            for h in range(2):
                nc.tensor.matmul(
                    P[:, h * 512 : (h + 1) * 512],
                    lhsT,
                    RT[:, h * 512 : (h + 1) * 512],
                    start=True, stop=True,
                )

        started = {}

        def mm2_pair(c1, c2, stop=False):
            """column-tiled mm2: chunk c1 -> band 0, chunk c2 -> band OB,
            running concurrently; both target halves. Each (half, band)
            group starts on its first write (start=True clears only that
            band's partition rows of the bank)."""
            for h in range(2):
                sl = slice(h * 512, (h + 1) * 512)
                for band, c in ((0, c1), (OB, c2)):
                    nc.tensor.matmul(
                        o2[band : band + DY + 1, sl],
                        YA[:, c, :],
                        THS[c % NTH][:, sl],
                        start=not started.get((h, band), False), stop=stop,
                        tile_position=(0, band),
                        # the sim's group-started map aliases the two
                        # bands of one bank; its per-row pending-zero
                        # value model is still exact
                        skip_group_check=True,
                    )
                    started[(h, band)] = True

        # ---- main loop over 64 context chunks: zs piece k rides chunk k
        # (gated by its own zc block); mm2 pairs start at chunk 18
        # (after the last zs mul releases o2), one pair per window ----
        next_pair = 0
        for j in range(NCHUNK):
            P = PSC[j % 3]
            score_mms(P, j)
            if j == NCHUNK - 1:
                # split the last exp so the final mm2 pair and the h0
                # evacuation start half a window earlier
                for h in range(2):
                    sl = slice(h * 512, (h + 1) * 512)
                    nc.scalar.activation(THS[j % NTH][:, sl], P[:, sl], Exp)
            elif j in DVE_EXP:
                # Schraudolph fast-exp on the (otherwise idle) DVE:
                # exp(x) ~ bitcast_f32(u32(x * 2^23/ln2 + (127*2^23 - C)));
                # ~3% max per-theta error on ~19% of contexts -> ~1.1e-2 on
                # the normalized output (validated in sim + offline), inside
                # the 2e-2 budget. Step 1 (the only PSUM reader) is emitted
                # for BOTH pair chunks before the remaining converts so the
                # score-tile rotation is released in time.
                e = j % 2
                nc.vector.tensor_scalar(
                    TMPA[e], P, SCH_A, SCH_B,
                    mybir.AluOpType.mult, mybir.AluOpType.add,
                )
                if j - 1 in DVE_EXP:                       # second of a pair
                    for ee, c in ((1 - e, j - 1), (e, j)):
                        nc.vector.tensor_copy(TMPB, TMPA[ee])
                        nc.vector.tensor_copy(THS[c % NTH], TMPB.bitcast(F32))
            else:
                nc.scalar.activation(THS[j % NTH], P, Exp)
            if 3 <= j <= NPIECE:
                zs_piece(j - 1)
            if j >= 18 and next_pair + 1 <= j - 1:
                mm2_pair(next_pair, next_pair + 1)
                next_pair += 2
            if j in (12, 28, 44):
                ya_piece({12: 1, 28: 2, 44: 3}[j], nc.vector.tensor_copy)

        # ---- epilogue: band 0 closed with chunk 62 (before chunk 63's
        # exp even finishes): DVE copies it and out0 rides the sync ring
        # immediately; band OB closes with chunk 63's mm2 and follows on
        # the ACT-copy + sync-ring path ----
        for h in range(2):
            sl = slice(h * 512, (h + 1) * 512)
            nc.tensor.matmul(
                o2[0 : DY + 1, sl], YA[:, 62, :], THS[62 % NTH][:, sl],
                start=False, stop=True, tile_position=(0, 0),
                skip_group_check=True,
            )
        nc.vector.tensor_copy(OSB[0 : DY + 1, :], o2[0 : DY + 1, :])
        nc.sync.dma_start(out=out0_d.ap(), in_=OSB[0 : DY + 1, :])
        for h in range(2):
            sl = slice(h * 512, (h + 1) * 512)
            nc.tensor.matmul(
                o2[OB : OB + DY + 1, sl], YA[:, 63, :], THS[63 % NTH][:, sl],
                start=False, stop=True, tile_position=(0, OB),
                skip_group_check=True,
            )
        nc.scalar.copy(OSB[OB : OB + DY + 1, :], o2[OB : OB + DY + 1, :])
        nc.sync.dma_start(out=out1_d.ap(), in_=OSB[OB : OB + DY + 1, :])


_CACHED = None


def _get_nc():
    global _CACHED
    if _CACHED is None:
        nc = bacc.Bacc(
            "TRN2",
            target_bir_lowering=False,
            debug=False,
            enable_asserts=False,
        )
        with tile.TileContext(nc) as tc:
            _build_kernel_body(tc)
        nc.compile()
        _CACHED = nc
    return _CACHED


def make_in_maps(z_context, y_context, z_target, W):
    """Host-side layout prep (transpose/reshape/cast only) + sharding."""
    z_context = np.asarray(z_context, dtype=np.float32)
    y_context = np.asarray(y_context, dtype=np.float32)
    z_target = np.asarray(z_target, dtype=np.float32)
    W = np.asarray(W, dtype=np.float32)

    zcT = np.ascontiguousarray(z_context.T.astype(np.float16))  # [64, 8192]
    zca = np.ascontiguousarray(zcT[:, 0:512])
    zcb = np.ascontiguousarray(zcT[:, 512:1024])
    zcv = np.ascontiguousarray(zcT[:, 1024:4608])
    zct = np.ascontiguousarray(zcT[:, 4608:8192])
    # chunk j partition p holds context j*128+p:
    # yck[p, j*DY+d] = y_context[j*128+p, d]
    yck = np.ascontiguousarray(
        y_context.reshape(NCHUNK, 128, DY).transpose(1, 0, 2).reshape(
            128, NCHUNK * DY
        )
    ).astype(ml_dtypes.bfloat16)
    wwt = np.ascontiguousarray(np.concatenate([W, W.T], axis=1))  # [64, 128]

    in_maps = []
    for i in range(NCORES):
        ztT = np.ascontiguousarray(
            z_target[i * TL : (i + 1) * TL].T.astype(np.float16)
        )
        m = {
            "wwt": wwt, "ztt": ztT, "yck": yck,
            "zca": zca, "zcb": zcb, "zcv": zcv, "zct": zct,
        }
        in_maps.append(m)
    return in_maps


def postprocess(results):
    """Gather per-core band outputs -> full (T, DY) normalized output."""
    outs = []
    for r in results:
        merged = (r["out0"] + r["out1"]).T                  # [TL, 33]
        outs.append(merged[:, :DY] / merged[:, DY : DY + 1])
    return np.concatenate(outs, axis=0).astype(np.float32)


def run(in_maps, **kwargs):
    nc = _get_nc()
    return run_bass_kernel_spmd(nc, in_maps, core_ids=list(range(NCORES)), **kwargs)


def kernel(z_context, y_context, z_target, W):
    in_maps = make_in_maps(z_context, y_context, z_target, W)
    res = run(in_maps)
    return postprocess(res.results)


# revision 21
# speedup vs baseline: 1.0526x; 1.0526x over previous
"""Trainium2 Bass kernel for nn_Interpolator (quadratic-form kernel interpolation).

Math (T=8192 targets, C=8192 contexts, D=64, DY=32):
    S = W + W^T
    scores[t,c] = (z_t - z_c)^T W (z_t - z_c)
                = q_tt[t] + q_cc[c] - z_t^T S z_c
    theta = exp(-scores);  out = (theta @ y_context) / theta.sum(-1, keepdim)

q_tt[t] scales whole theta rows and cancels in the normalization -> dropped.
q_cc[c] = 0.5 * z_c^T S z_c is folded into the main matmul contraction:
the stationary operand LC has 128 rows: rows 0..63 = zc^T, rows 64..127 =
0.5*(zc .* (S zc)); the moving operand RT has rows 0..63 = S^T z_t and
rows 64..127 = -1. A single K=128 fp16 matmul then yields cross - q_cc
directly (matmul cost depends only on moving columns, so the fold is free).

Sharding: data-parallel over targets; each of the 8 cores takes T/8 = 1024
targets and the full context set.

Per-core device program (final). The ACT exp stream is the roofline:
measured, back-to-back 1024-wide ACTIVATEs run at ~996 ns each (pipe-fill
overlaps), beating any wider-instruction scheme the PSUM bank budget
allows. The loop is a 3-tile rotation of [128,1024] score tiles (6 banks)
whose exps run back-to-back; nothing else touches those tiles mid-loop:
  - every 4th chunk's exp runs on the otherwise-idle DVE instead via
    Schraudolph fast-exp (tensor_scalar mul+add, u32 convert, bitcast
    copy to bf16), trimming the ACT stream by ~1us per offloaded chunk
    (~3% per-theta error on those contexts; ~1.1e-2 on the normalized
    output vs the 2e-2 budget, validated in sim and offline).
  - the 16 zs pieces (LC rows 64..127): 0-1 staged in the rotation tiles
    pre-loop; piece k rides chunk k+1 staged in the o2 banks' rows 0:64
    (free until mm2 starts), DVE writing LC directly - no coupling into
    the exp stream.
  - mm2 starts at chunk 18: theta tiles live in a 20-deep ring, and
    column-tiled PAIRS (chunk c -> band 0 = partitions 0:33, chunk c+1 ->
    band 64:97, concurrent in PE column-groups) catch up one pair per
    window. The bands are summed on the host (no cross-partition add on
    device).
  - input DMAs ride the two hardware DGE rings in first-use order:
    scalar ring W -> zc[0:512] -> y h0; sync ring zt -> zc[512:1024] ->
    zc bulk -> y h1 (ring wake-up is ~2-4us, so the critical chain is
    split across both). Dep-free filler matmuls into o2 keep PE busy
    through the DMA phase (HAM 8/8 before chunk 0).
  - RT = S^T zt is staged in PS2 and cast by two ACT copies (ACT is idle
    pre-loop; engine writes to one tile serialize anyway).
  - output: one [33, TL] DMA per band per ring (33 rows fan across 11
    DMA engines; a 97-row transfer lands on ONE engine - measured);
    chunk 63's exp is split 2x512, band 0 closes with chunk 62's mm2 so
    its copy and DMA overlap the last window.
Host: shard/transpose/cast inputs (layout only); gather, sum the two
bands, divide numerator rows by the denominator row.
"""

import ml_dtypes
import numpy as np

import concourse.bacc as bacc
import concourse.bass as bass
import concourse.mybir as mybir
import concourse.tile as tile
from concourse.bass_utils import run_bass_kernel_spmd

F32 = mybir.dt.float32
F16 = mybir.dt.float16
BF16 = mybir.dt.bfloat16

T, C, D, DY = 8192, 8192, 64, 32
NCORES = 8
TL = T // NCORES          # 1024 targets per core
NCHUNK = C // 128         # 64 context chunks of 128
NPIECE = C // 512         # 16 zs pieces of 512 contexts
NWARM = 4
NFILL = 6
NTH = 20                  # theta ring depth (max mm2 lag 16 + safety)
OB = 64                   # second mm2 band base partition (column-group 2)
DVE_EXP = set(range(20, 61, 4))   # chunks whose exp runs on the DVE
SCH_A = float(2 ** 23 / np.log(2))          # Schraudolph scale
SCH_B = float(127 * 2 ** 23 - 486411)       # Schraudolph bias (minimax C)


def _build_kernel_body(tc: tile.TileContext):
    nc = tc.nc
    Exp = mybir.ActivationFunctionType.Exp

    wwt_d = nc.dram_tensor("wwt", [D, 2 * D], F32, kind="ExternalInput")
    zt_d = nc.dram_tensor("ztt", [D, TL], F16, kind="ExternalInput")
    zca_d = nc.dram_tensor("zca", [D, 512], F16, kind="ExternalInput")
    zcb_d = nc.dram_tensor("zcb", [D, 512], F16, kind="ExternalInput")
    zcv_d = nc.dram_tensor("zcv", [D, 3584], F16, kind="ExternalInput")
    zct_d = nc.dram_tensor("zct", [D, 3584], F16, kind="ExternalInput")
    y_d = nc.dram_tensor("yck", [128, NCHUNK * DY], BF16, kind="ExternalInput")
    out0_d = nc.dram_tensor("out0", [DY + 1, TL], F32, kind="ExternalOutput")
    out1_d = nc.dram_tensor("out1", [DY + 1, TL], F32, kind="ExternalOutput")

    with (
        tc.tile_pool(name="sb", bufs=1) as sb,
        tc.tile_pool(name="pp", bufs=1, space="PSUM") as pp,
    ):
        # ---- resident SBUF slabs ----
        LC = sb.tile([128, C], F16, name="lc")
        RT = sb.tile([128, TL], F16, name="rt")
        ZT = sb.tile([D, TL], F16, name="zt")
        YT = sb.tile([128, NCHUNK * DY], BF16, name="yt")
        YA = sb.tile([128, NCHUNK, DY + 1], BF16, name="ya")
        WW = sb.tile([D, 2 * D], F32, name="ww")
        SS = sb.tile([D, D], F16, name="ss")
        SSH = sb.tile([D, D], F16, name="ssh")
        THS = [sb.tile([128, TL], BF16, name=f"th{i}") for i in range(NTH)]
        OSB = sb.tile([OB + DY + 1, TL], F32, name="osb")
        WRM = sb.tile([128, 512], BF16, name="wrm")
        TMPA = [sb.tile([128, TL], F32, name=f"tmpa{i}") for i in range(2)]
        TMPB = sb.tile([128, TL], mybir.dt.uint32, name="tmpb")
        EXD = sb.tile([D, 1], F32, name="exd")

        # ---- PSUM: 3 rotating score tiles (6 banks) + o2 (2 banks) ----
        PS0 = pp.tile([128, TL], F32, tag="ring0", name="ps0")
        PS1 = pp.tile([128, TL], F32, tag="ring1", name="ps1")
        PS2 = pp.tile([128, TL], F32, tag="ring2", name="ps2")
        o2 = pp.tile([128, TL], F32, tag="o2", name="o2")
        PSC = [PS0, PS1, PS2]

        # ---- input DMAs on the two HWDGE rings, first-use order ----
        half_y = NCHUNK * DY // 2
        nc.scalar.dma_start(out=WW, in_=wwt_d.ap())
        nc.scalar.dma_start(out=LC[:D, 0:512], in_=zca_d.ap())
        nc.scalar.dma_start(out=YT[:, :half_y], in_=y_d.ap()[:, :half_y])
        nc.sync.dma_start(out=ZT, in_=zt_d.ap())
        nc.sync.dma_start(out=LC[:D, 512:1024], in_=zcb_d.ap())
        nc.sync.dma_start(out=LC[:D, 1024:4608], in_=zcv_d.ap())
        nc.sync.dma_start(out=LC[:D, 4608:8192], in_=zct_d.ap())
        nc.sync.dma_start(out=YT[:, half_y:], in_=y_d.ap()[:, half_y:])

        # exp-table preload (so the first chunk ACTIVATE is cheap)
        nc.vector.memset(EXD, 0.0)
        nc.scalar.activation(EXD, EXD, Exp)

        # ---- PE warm-up + fillers: keep PE busy through the DMA phase
        # (HAM -> 8/8); the scheduler slots real matmuls in as ready ----
        nc.vector.memset(WRM, 0.5)
        for i in range(NWARM):
            nc.tensor.matmul(
                PSC[i % 2][:, (i // 2) * 512 : (i // 2 + 1) * 512],
                WRM[:, 0:128], WRM, start=True, stop=True,
            )
        for i in range(NFILL):
            nc.tensor.matmul(
                o2[:, (i % 2) * 512 : (i % 2 + 1) * 512], WRM[:, 0:128], WRM,
                start=True, stop=True,
            )

        # ---- DVE prelude chain ----
        nc.vector.memset(RT[D:128, :], -1.0)
        nc.vector.tensor_add(SS, WW[:, 0:D], WW[:, D : 2 * D])   # fp16 S
        nc.vector.tensor_scalar_mul(SSH, SS, 0.5)                # fp16 S/2

        # ---- RT rows 0..63 = S^T zt staged in PS2, one ACT cast (ACT is
        # idle pre-loop); chunks 0/1 never wait on PS2 ----
        for h in range(2):
            sl = slice(h * 512, (h + 1) * 512)
            nc.tensor.matmul(PS2[:D, sl], SS, ZT[:, sl], start=True, stop=True)
        nc.scalar.copy(RT[:D, 0:512], PS2[:D, 0:512])
        nc.scalar.copy(RT[:D, 512:1024], PS2[:D, 512:1024])

        # ---- zs pieces 0,1 staged in PS0/PS1 region [512:1024]; DVE
        # writes LC rows 64:128 directly ----
        for k in range(2):
            sl = slice(512 * k, 512 * (k + 1))
            nc.tensor.matmul(PSC[k][:D, 512:1024], SSH, LC[:D, sl],
                             start=True, stop=True)
            nc.vector.tensor_mul(LC[D:128, sl], PSC[k][:D, 512:1024],
                                 LC[:D, sl])

        # zs piece k (2..15) staged in o2 rows 0:64, emitted at chunk
        # k-2 so the alternating-region WAR chain always has two windows
        # of slack and never head-of-line-blocks the PE queue; DVE
        # writes LC rows 64:128 directly (consumers are 8+ windows out)
        def zs_piece(k):
            sl = slice(512 * k, 512 * (k + 1))
            st = slice((k % 2) * 512, (k % 2 + 1) * 512)
            nc.tensor.matmul(o2[:D, st], SSH, LC[:D, sl], start=True,
                             stop=True)
            nc.vector.tensor_mul(LC[D:128, sl], o2[:D, st], LC[:D, sl])

        # y_aug piece q (16 chunks): [128, 16, 33]; col 32 = 1.0
        nc.vector.memset(YA[:, :, DY : DY + 1], 1.0)
        qy = NCHUNK // 4 * DY

        def ya_piece(q, eng):
            eng(
                YA[:, q * 16 : (q + 1) * 16, 0:DY],
                YT[:, q * qy : (q + 1) * qy].rearrange("p (j d) -> p j d", d=DY),
            )

        ya_piece(0, nc.vector.tensor_copy)   # DVE; ACT stays on the cast chain

        def score_mms(P, c):
            lhsT = LC[:, c * 128 : (c + 1) * 128]
            for h in range(2):
                nc.tensor.matmul(
                    P[:, h * 512 : (h + 1) * 512],
                    lhsT,
                    RT[:, h * 512 : (h + 1) * 512],
                    start=True, stop=True,
                )

        started = {}

        def mm2_pair(c1, c2, stop=False):
            """column-tiled mm2: chunk c1 -> band 0, chunk c2 -> band OB,
            running concurrently; both target halves. Each (half, band)
            group starts on its first write (start=True clears only that
            band's partition rows of the bank)."""
            for h in range(2):
                sl = slice(h * 512, (h + 1) * 512)
                for band, c in ((0, c1), (OB, c2)):
                    nc.tensor.matmul(
                        o2[band : band + DY + 1, sl],
                        YA[:, c, :],
                        THS[c % NTH][:, sl],
                        start=not started.get((h, band), False), stop=stop,
                        tile_position=(0, band),
                        # the sim's group-started map aliases the two
                        # bands of one bank; its per-row pending-zero
                        # value model is still exact
                        skip_group_check=True,
                    )
                    started[(h, band)] = True

        # ---- main loop over 64 context chunks: zs piece k rides chunk k
        # (gated by its own zc block); mm2 pairs start at chunk 18
        # (after the last zs mul releases o2), one pair per window ----
        next_pair = 0
        for j in range(NCHUNK):
            P = PSC[j % 3]
            score_mms(P, j)
            if j == NCHUNK - 1:
                # split the last exp so the final mm2 pair and the h0
                # evacuation start half a window earlier
                for h in range(2):
                    sl = slice(h * 512, (h + 1) * 512)
                    nc.scalar.activation(THS[j % NTH][:, sl], P[:, sl], Exp)
            elif j in DVE_EXP:
                # Schraudolph fast-exp on the (otherwise idle) DVE:
                # exp(x) ~ bitcast_f32(u32(x * 2^23/ln2 + (127*2^23 - C)));
                # ~3% max per-theta error on ~17% of contexts -> ~1.1e-2 on
                # the normalized output (validated in sim + offline), inside
                # the 2e-2 budget. Frees ~1us of ACT stream per chunk.
                nc.vector.tensor_scalar(
                    TMPA[0], P, SCH_A, SCH_B,
                    mybir.AluOpType.mult, mybir.AluOpType.add,
                )
                nc.vector.tensor_copy(TMPB, TMPA[0])       # f32 -> u32 convert
                nc.vector.tensor_copy(THS[j % NTH], TMPB.bitcast(F32))
            else:
                nc.scalar.activation(THS[j % NTH], P, Exp)
            if 3 <= j <= NPIECE:
                zs_piece(j - 1)
            if j >= 18 and next_pair + 1 <= j - 1:
                mm2_pair(next_pair, next_pair + 1)
                next_pair += 2
            if j in (12, 28, 44):
                ya_piece({12: 1, 28: 2, 44: 3}[j], nc.vector.tensor_copy)

        # ---- epilogue: band 0 closed with chunk 62 (before chunk 63's
        # exp even finishes): DVE copies it and out0 rides the sync ring
        # immediately; band OB closes with chunk 63's mm2 and follows on
        # the ACT-copy + sync-ring path ----
        for h in range(2):
            sl = slice(h * 512, (h + 1) * 512)
            nc.tensor.matmul(
                o2[0 : DY + 1, sl], YA[:, 62, :], THS[62 % NTH][:, sl],
                start=False, stop=True, tile_position=(0, 0),
                skip_group_check=True,
            )
        nc.vector.tensor_copy(OSB[0 : DY + 1, :], o2[0 : DY + 1, :])
        nc.sync.dma_start(out=out0_d.ap(), in_=OSB[0 : DY + 1, :])
        for h in range(2):
            sl = slice(h * 512, (h + 1) * 512)
            nc.tensor.matmul(
                o2[OB : OB + DY + 1, sl], YA[:, 63, :], THS[63 % NTH][:, sl],
                start=False, stop=True, tile_position=(0, OB),
                skip_group_check=True,
            )
        nc.scalar.copy(OSB[OB : OB + DY + 1, :], o2[OB : OB + DY + 1, :])
        nc.sync.dma_start(out=out1_d.ap(), in_=OSB[OB : OB + DY + 1, :])


_CACHED = None


def _get_nc():
    global _CACHED
    if _CACHED is None:
        nc = bacc.Bacc(
            "TRN2",
            target_bir_lowering=False,
            debug=False,
            enable_asserts=False,
        )
        with tile.TileContext(nc) as tc:
            _build_kernel_body(tc)
        nc.compile()
        _CACHED = nc
    return _CACHED


def make_in_maps(z_context, y_context, z_target, W):
    """Host-side layout prep (transpose/reshape/cast only) + sharding."""
    z_context = np.asarray(z_context, dtype=np.float32)
    y_context = np.asarray(y_context, dtype=np.float32)
    z_target = np.asarray(z_target, dtype=np.float32)
    W = np.asarray(W, dtype=np.float32)

    zcT = np.ascontiguousarray(z_context.T.astype(np.float16))  # [64, 8192]
    zca = np.ascontiguousarray(zcT[:, 0:512])
    zcb = np.ascontiguousarray(zcT[:, 512:1024])
    zcv = np.ascontiguousarray(zcT[:, 1024:4608])
    zct = np.ascontiguousarray(zcT[:, 4608:8192])
    # chunk j partition p holds context j*128+p:
    # yck[p, j*DY+d] = y_context[j*128+p, d]
    yck = np.ascontiguousarray(
        y_context.reshape(NCHUNK, 128, DY).transpose(1, 0, 2).reshape(
            128, NCHUNK * DY
        )
    ).astype(ml_dtypes.bfloat16)
    wwt = np.ascontiguousarray(np.concatenate([W, W.T], axis=1))  # [64, 128]

    in_maps = []
    for i in range(NCORES):
        ztT = np.ascontiguousarray(
            z_target[i * TL : (i + 1) * TL].T.astype(np.float16)
        )
        m = {
            "wwt": wwt, "ztt": ztT, "yck": yck,
            "zca": zca, "zcb": zcb, "zcv": zcv, "zct": zct,
        }
        in_maps.append(m)
    return in_maps


def postprocess(results):
    """Gather per-core band outputs -> full (T, DY) normalized output."""
    outs = []
    for r in results:
        merged = (r["out0"] + r["out1"]).T                  # [TL, 33]
        outs.append(merged[:, :DY] / merged[:, DY : DY + 1])
    return np.concatenate(outs, axis=0).astype(np.float32)


def run(in_maps, **kwargs):
    nc = _get_nc()
    return run_bass_kernel_spmd(nc, in_maps, core_ids=list(range(NCORES)), **kwargs)


def kernel(z_context, y_context, z_target, W):
    in_maps = make_in_maps(z_context, y_context, z_target, W)
    res = run(in_maps)
    return postprocess(res.results)


# revision 22
# speedup vs baseline: 1.1370x; 1.0802x over previous
"""Trainium2 Bass kernel for nn_Interpolator (quadratic-form kernel interpolation).

Math (T=8192 targets, C=8192 contexts, D=64, DY=32):
    S = W + W^T
    scores[t,c] = (z_t - z_c)^T W (z_t - z_c)
                = q_tt[t] + q_cc[c] - z_t^T S z_c
    theta = exp(-scores);  out = (theta @ y_context) / theta.sum(-1, keepdim)

q_tt[t] scales whole theta rows and cancels in the normalization -> dropped.
q_cc[c] = 0.5 * z_c^T S z_c is folded into the main matmul contraction:
the stationary operand LC has 128 rows: rows 0..63 = zc^T, rows 64..127 =
0.5*(zc .* (S zc)); the moving operand RT has rows 0..63 = S^T z_t and
rows 64..127 = -1. A single K=128 fp16 matmul then yields cross - q_cc
directly (matmul cost depends only on moving columns, so the fold is free).

Sharding: data-parallel over targets; each of the 8 cores takes T/8 = 1024
targets and the full context set.

Per-core device program (final). The ACT exp stream is the roofline:
measured, back-to-back 1024-wide ACTIVATEs run at ~996 ns each (pipe-fill
overlaps), beating any wider-instruction scheme the PSUM bank budget
allows. The loop is a 3-tile rotation of [128,1024] score tiles (6 banks)
whose exps run back-to-back; nothing else touches those tiles mid-loop:
  - every 4th chunk's exp runs on the otherwise-idle DVE instead via
    Schraudolph fast-exp (tensor_scalar mul+add, u32 convert, bitcast
    copy to bf16), trimming the ACT stream by ~1us per offloaded chunk
    (~3% per-theta error on those contexts; ~1.1e-2 on the normalized
    output vs the 2e-2 budget, validated in sim and offline).
  - the 16 zs pieces (LC rows 64..127): 0-1 staged in the rotation tiles
    pre-loop; piece k rides chunk k+1 staged in the o2 banks' rows 0:64
    (free until mm2 starts), DVE writing LC directly - no coupling into
    the exp stream.
  - mm2 starts at chunk 18: theta tiles live in a 20-deep ring, and
    column-tiled PAIRS (chunk c -> band 0 = partitions 0:33, chunk c+1 ->
    band 64:97, concurrent in PE column-groups) catch up one pair per
    window. The bands are summed on the host (no cross-partition add on
    device).
  - input DMAs ride the two hardware DGE rings in first-use order:
    scalar ring W -> zc[0:512] -> y h0; sync ring zt -> zc[512:1024] ->
    zc bulk -> y h1 (ring wake-up is ~2-4us, so the critical chain is
    split across both). Dep-free filler matmuls into o2 keep PE busy
    through the DMA phase (HAM 8/8 before chunk 0).
  - RT = S^T zt is staged in PS2 and cast by two ACT copies (ACT is idle
    pre-loop; engine writes to one tile serialize anyway).
  - output: one [33, TL] DMA per band per ring (33 rows fan across 11
    DMA engines; a 97-row transfer lands on ONE engine - measured);
    chunk 63's exp is split 2x512, band 0 closes with chunk 62's mm2 so
    its copy and DMA overlap the last window.
Host: shard/transpose/cast inputs (layout only); gather, sum the two
bands, divide numerator rows by the denominator row.
"""

import ml_dtypes
import numpy as np

import concourse.bacc as bacc
import concourse.bass as bass
import concourse.mybir as mybir
import concourse.tile as tile
from concourse.bass_utils import run_bass_kernel_spmd

F32 = mybir.dt.float32
F16 = mybir.dt.float16
BF16 = mybir.dt.bfloat16

T, C, D, DY = 8192, 8192, 64, 32
NCORES = 8
TL = T // NCORES          # 1024 targets per core
NCHUNK = C // 128         # 64 context chunks of 128
NPIECE = C // 512         # 16 zs pieces of 512 contexts
NWARM = 4
NFILL = 6
NTH = 20                  # theta ring depth (max mm2 lag 16 + safety)
OB = 64                   # second mm2 band base partition (column-group 2)
DVE_EXP = set(range(20, 61, 4))   # chunks whose exp runs on the DVE
SCH_A = float(2 ** 23 / np.log(2))          # Schraudolph scale
SCH_B = float(127 * 2 ** 23 - 486411)       # Schraudolph bias (minimax C)


def _build_kernel_body(tc: tile.TileContext):
    nc = tc.nc
    Exp = mybir.ActivationFunctionType.Exp

    wwt_d = nc.dram_tensor("wwt", [D, 2 * D], F32, kind="ExternalInput")
    zt_d = nc.dram_tensor("ztt", [D, TL], F16, kind="ExternalInput")
    zca_d = nc.dram_tensor("zca", [D, 512], F16, kind="ExternalInput")
    zcb_d = nc.dram_tensor("zcb", [D, 512], F16, kind="ExternalInput")
    zcv_d = nc.dram_tensor("zcv", [D, 3584], F16, kind="ExternalInput")
    zct_d = nc.dram_tensor("zct", [D, 3584], F16, kind="ExternalInput")
    y_d = nc.dram_tensor("yck", [128, NCHUNK * DY], BF16, kind="ExternalInput")
    out0_d = nc.dram_tensor("out0", [DY + 1, TL], F32, kind="ExternalOutput")
    out1_d = nc.dram_tensor("out1", [DY + 1, TL], F32, kind="ExternalOutput")

    with (
        tc.tile_pool(name="sb", bufs=1) as sb,
        tc.tile_pool(name="pp", bufs=1, space="PSUM") as pp,
    ):
        # ---- resident SBUF slabs ----
        LC = sb.tile([128, C], F16, name="lc")
        RT = sb.tile([128, TL], F16, name="rt")
        ZT = sb.tile([D, TL], F16, name="zt")
        YT = sb.tile([128, NCHUNK * DY], BF16, name="yt")
        YA = sb.tile([128, NCHUNK, DY + 1], BF16, name="ya")
        WW = sb.tile([D, 2 * D], F32, name="ww")
        SS = sb.tile([D, D], F16, name="ss")
        SSH = sb.tile([D, D], F16, name="ssh")
        THS = [sb.tile([128, TL], BF16, name=f"th{i}") for i in range(NTH)]
        OSB = sb.tile([OB + DY + 1, TL], F32, name="osb")
        WRM = sb.tile([128, 512], BF16, name="wrm")
        TMPA = [sb.tile([128, TL], F32, name=f"tmpa{i}") for i in range(2)]
        TMPB = sb.tile([128, TL], mybir.dt.uint32, name="tmpb")
        EXD = sb.tile([D, 1], F32, name="exd")

        # ---- PSUM: 3 rotating score tiles (6 banks) + o2 (2 banks) ----
        PS0 = pp.tile([128, TL], F32, tag="ring0", name="ps0")
        PS1 = pp.tile([128, TL], F32, tag="ring1", name="ps1")
        PS2 = pp.tile([128, TL], F32, tag="ring2", name="ps2")
        o2 = pp.tile([128, TL], F32, tag="o2", name="o2")
        PSC = [PS0, PS1, PS2]

        # ---- input DMAs on the two HWDGE rings, first-use order ----
        half_y = NCHUNK * DY // 2
        nc.scalar.dma_start(out=WW, in_=wwt_d.ap())
        nc.scalar.dma_start(out=LC[:D, 0:512], in_=zca_d.ap())
        nc.scalar.dma_start(out=YT[:, :half_y], in_=y_d.ap()[:, :half_y])
        nc.sync.dma_start(out=ZT, in_=zt_d.ap())
        nc.sync.dma_start(out=LC[:D, 512:1024], in_=zcb_d.ap())
        nc.sync.dma_start(out=LC[:D, 1024:4608], in_=zcv_d.ap())
        nc.sync.dma_start(out=LC[:D, 4608:8192], in_=zct_d.ap())
        nc.sync.dma_start(out=YT[:, half_y:], in_=y_d.ap()[:, half_y:])

        # exp-table preload (so the first chunk ACTIVATE is cheap)
        nc.vector.memset(EXD, 0.0)
        nc.scalar.activation(EXD, EXD, Exp)

        # ---- PE warm-up + fillers: keep PE busy through the DMA phase
        # (HAM -> 8/8); the scheduler slots real matmuls in as ready ----
        nc.vector.memset(WRM, 0.5)
        for i in range(NWARM):
            nc.tensor.matmul(
                PSC[i % 2][:, (i // 2) * 512 : (i // 2 + 1) * 512],
                WRM[:, 0:128], WRM, start=True, stop=True,
            )
        for i in range(NFILL):
            nc.tensor.matmul(
                o2[:, (i % 2) * 512 : (i % 2 + 1) * 512], WRM[:, 0:128], WRM,
                start=True, stop=True,
            )

        # ---- DVE prelude chain ----
        nc.vector.memset(RT[D:128, :], -1.0)
        nc.vector.tensor_add(SS, WW[:, 0:D], WW[:, D : 2 * D])   # fp16 S
        nc.vector.tensor_scalar_mul(SSH, SS, 0.5)                # fp16 S/2

        # ---- RT rows 0..63 = S^T zt staged in PS2, one ACT cast (ACT is
        # idle pre-loop); chunks 0/1 never wait on PS2 ----
        for h in range(2):
            sl = slice(h * 512, (h + 1) * 512)
            nc.tensor.matmul(PS2[:D, sl], SS, ZT[:, sl], start=True, stop=True)
        nc.scalar.copy(RT[:D, 0:512], PS2[:D, 0:512])
        nc.scalar.copy(RT[:D, 512:1024], PS2[:D, 512:1024])

        # ---- zs pieces 0,1 staged in PS0/PS1 region [512:1024]; DVE
        # writes LC rows 64:128 directly ----
        for k in range(2):
            sl = slice(512 * k, 512 * (k + 1))
            nc.tensor.matmul(PSC[k][:D, 512:1024], SSH, LC[:D, sl],
                             start=True, stop=True)
            nc.vector.tensor_mul(LC[D:128, sl], PSC[k][:D, 512:1024],
                                 LC[:D, sl])

        # zs piece k (2..15) staged in o2 rows 0:64, emitted at chunk
        # k-2 so the alternating-region WAR chain always has two windows
        # of slack and never head-of-line-blocks the PE queue; DVE
        # writes LC rows 64:128 directly (consumers are 8+ windows out)
        def zs_piece(k):
            sl = slice(512 * k, 512 * (k + 1))
            st = slice((k % 2) * 512, (k % 2 + 1) * 512)
            nc.tensor.matmul(o2[:D, st], SSH, LC[:D, sl], start=True,
                             stop=True)
            nc.vector.tensor_mul(LC[D:128, sl], o2[:D, st], LC[:D, sl])

        # y_aug piece q (16 chunks): [128, 16, 33]; col 32 = 1.0
        nc.vector.memset(YA[:, :, DY : DY + 1], 1.0)
        qy = NCHUNK // 4 * DY

        def ya_piece(q, eng):
            eng(
                YA[:, q * 16 : (q + 1) * 16, 0:DY],
                YT[:, q * qy : (q + 1) * qy].rearrange("p (j d) -> p j d", d=DY),
            )

        ya_piece(0, nc.vector.tensor_copy)   # DVE; ACT stays on the cast chain

        def score_mms(P, c):
            lhsT = LC[:, c * 128 : (c + 1) * 128]
            for h in range(2):
                nc.tensor.matmul(
                    P[:, h * 512 : (h + 1) * 512],
                    lhsT,
                    RT[:, h * 512 : (h + 1) * 512],
                    start=True, stop=True,
                )

        started = {}

        def mm2_pair(c1, c2, stop=False):
            """column-tiled mm2: chunk c1 -> band 0, chunk c2 -> band OB,
            running concurrently; both target halves. Each (half, band)
            group starts on its first write (start=True clears only that
            band's partition rows of the bank)."""
            for h in range(2):
                sl = slice(h * 512, (h + 1) * 512)
                for band, c in ((0, c1), (OB, c2)):
                    nc.tensor.matmul(
                        o2[band : band + DY + 1, sl],
                        YA[:, c, :],
                        THS[c % NTH][:, sl],
                        start=not started.get((h, band), False), stop=stop,
                        tile_position=(0, band),
                        # the sim's group-started map aliases the two
                        # bands of one bank; its per-row pending-zero
                        # value model is still exact
                        skip_group_check=True,
                    )
                    started[(h, band)] = True

        # ---- main loop over 64 context chunks: zs piece k rides chunk k
        # (gated by its own zc block); mm2 pairs start at chunk 18
        # (after the last zs mul releases o2), one pair per window ----
        next_pair = 0
        for j in range(NCHUNK):
            P = PSC[j % 3]
            score_mms(P, j)
            if j == NCHUNK - 1:
                # split the last exp so the final mm2 pair and the h0
                # evacuation start half a window earlier
                for h in range(2):
                    sl = slice(h * 512, (h + 1) * 512)
                    nc.scalar.activation(THS[j % NTH][:, sl], P[:, sl], Exp)
            elif j in DVE_EXP:
                # Schraudolph fast-exp on the (otherwise idle) DVE:
                # exp(x) ~ bitcast_f32(u32(x * 2^23/ln2 + (127*2^23 - C)));
                # ~3% max per-theta error on ~17% of contexts -> ~1.1e-2 on
                # the normalized output (validated in sim + offline), inside
                # the 2e-2 budget. Frees ~1us of ACT stream per chunk.
                nc.vector.tensor_scalar(
                    TMPA[0], P, SCH_A, SCH_B,
                    mybir.AluOpType.mult, mybir.AluOpType.add,
                )
                nc.vector.tensor_copy(TMPB, TMPA[0])       # f32 -> u32 convert
                nc.vector.tensor_copy(THS[j % NTH], TMPB.bitcast(F32))
            else:
                nc.scalar.activation(THS[j % NTH], P, Exp)
            if 3 <= j <= NPIECE:
                zs_piece(j - 1)
            if j >= 18 and next_pair + 1 <= j - 3:
                # lag >= 3: a pair must never read the theta tile of the
                # immediately preceding exp, or the PE FIFO serializes the
                # next score matmuls behind that exp (binds around the
                # DVE-offloaded chunks' gaps)
                mm2_pair(next_pair, next_pair + 1)
                next_pair += 2
            if j in (12, 28, 44):
                ya_piece({12: 1, 28: 2, 44: 3}[j], nc.vector.tensor_copy)

        # ---- epilogue: band 0 closed with chunk 62 (before chunk 63's
        # exp even finishes): DVE copies it and out0 rides the sync ring
        # immediately; band OB closes with chunk 63's mm2 and follows on
        # the ACT-copy + sync-ring path ----
        mm2_pair(60, 61)
        for h in range(2):
            sl = slice(h * 512, (h + 1) * 512)
            nc.tensor.matmul(
                o2[0 : DY + 1, sl], YA[:, 62, :], THS[62 % NTH][:, sl],
                start=False, stop=True, tile_position=(0, 0),
                skip_group_check=True,
            )
        nc.vector.tensor_copy(OSB[0 : DY + 1, :], o2[0 : DY + 1, :])
        nc.sync.dma_start(out=out0_d.ap(), in_=OSB[0 : DY + 1, :])
        for h in range(2):
            sl = slice(h * 512, (h + 1) * 512)
            nc.tensor.matmul(
                o2[OB : OB + DY + 1, sl], YA[:, 63, :], THS[63 % NTH][:, sl],
                start=False, stop=True, tile_position=(0, OB),
                skip_group_check=True,
            )
        nc.scalar.copy(OSB[OB : OB + DY + 1, :], o2[OB : OB + DY + 1, :])
        nc.sync.dma_start(out=out1_d.ap(), in_=OSB[OB : OB + DY + 1, :])


_CACHED = None


def _get_nc():
    global _CACHED
    if _CACHED is None:
        nc = bacc.Bacc(
            "TRN2",
            target_bir_lowering=False,
            debug=False,
            enable_asserts=False,
        )
        with tile.TileContext(nc) as tc:
            _build_kernel_body(tc)
        nc.compile()
        _CACHED = nc
    return _CACHED


def make_in_maps(z_context, y_context, z_target, W):
    """Host-side layout prep (transpose/reshape/cast only) + sharding."""
    z_context = np.asarray(z_context, dtype=np.float32)
    y_context = np.asarray(y_context, dtype=np.float32)
    z_target = np.asarray(z_target, dtype=np.float32)
    W = np.asarray(W, dtype=np.float32)

    zcT = np.ascontiguousarray(z_context.T.astype(np.float16))  # [64, 8192]
    zca = np.ascontiguousarray(zcT[:, 0:512])
    zcb = np.ascontiguousarray(zcT[:, 512:1024])
    zcv = np.ascontiguousarray(zcT[:, 1024:4608])
    zct = np.ascontiguousarray(zcT[:, 4608:8192])
    # chunk j partition p holds context j*128+p:
    # yck[p, j*DY+d] = y_context[j*128+p, d]
    yck = np.ascontiguousarray(
        y_context.reshape(NCHUNK, 128, DY).transpose(1, 0, 2).reshape(
            128, NCHUNK * DY
        )
    ).astype(ml_dtypes.bfloat16)
    wwt = np.ascontiguousarray(np.concatenate([W, W.T], axis=1))  # [64, 128]

    in_maps = []
    for i in range(NCORES):
        ztT = np.ascontiguousarray(
            z_target[i * TL : (i + 1) * TL].T.astype(np.float16)
        )
        m = {
            "wwt": wwt, "ztt": ztT, "yck": yck,
            "zca": zca, "zcb": zcb, "zcv": zcv, "zct": zct,
        }
        in_maps.append(m)
    return in_maps


def postprocess(results):
    """Gather per-core band outputs -> full (T, DY) normalized output."""
    outs = []
    for r in results:
        merged = (r["out0"] + r["out1"]).T                  # [TL, 33]
        outs.append(merged[:, :DY] / merged[:, DY : DY + 1])
    return np.concatenate(outs, axis=0).astype(np.float32)


def run(in_maps, **kwargs):
    nc = _get_nc()
    return run_bass_kernel_spmd(nc, in_maps, core_ids=list(range(NCORES)), **kwargs)


def kernel(z_context, y_context, z_target, W):
    in_maps = make_in_maps(z_context, y_context, z_target, W)
    res = run(in_maps)
    return postprocess(res.results)
